# revision 2
# baseline (speedup 1.0000x reference)
"""Dense-MoE (top-2 of 8 experts) TRN2 kernel v4: expert-parallel, bf16 matmuls.

Host side: softmax + top-2 routing, per-expert token gather (padded to the max
expert load), weight re-layout + bf16 conversion. Device side (per core = one
expert), all matmul operands bf16, PSUM accumulation f32:
    phase A:  h[f, c] = silu(gw @ x) * (uw @ x)      [f-major, bf16 in SBUF]
    phase B:  outT[d, c] = sum_f dwT[f, d] * h[f, c]  [tokens on the free dim]
The routing weight and the scatter-add back to [T, D] happen on the host
(out[t] += w_t * outT[:, c].T), so the device kernel needs no tw input.

vs v2: gate/up matmuls interleave per d-slice so the first f-tile's x
streaming window doubles (the gate-only pass needs 2.9 MB in 4.3 us, over
HBM rate; interleaved it is 3.1 MB in 8.6 us); the startup DMAs issue in PE
consumption order as (gw 2-slice, uw 2-slice, x 2-slice) groups, all on the
Sync queue (GpSimd-issued DMAs land on a software-dynamic DGE queue with
~1.6 us drains -- tried in v4, slower); phase B PSUM banks alternate tag
sets so consecutive d-tiles don't wait on the drain, and the drain copies
alternate scalar/vector so the final d-tile empties ~2 us sooner. fp16 was
tried and is ~25% slower than bf16 on the PE despite the cost model
claiming parity.
"""
import sys

sys.path.insert(0, "/opt/trn_rl_repo")

import ml_dtypes
import numpy as np

import concourse.bass as bass
from concourse import bacc
import concourse.mybir as mybir
import concourse.tile as tile
from concourse.bass_utils import run_bass_kernel_spmd
from concourse.bass import ds

T, D, F, E, TOPK = 4096, 1024, 2048, 8, 2
P = 128
N_CORES = 8

F32 = mybir.dt.float32
BF16 = mybir.dt.bfloat16
BF = ml_dtypes.bfloat16


def _chunks(cap, n):
    """Split [0, cap) into n even-sized chunks (sizes even, ~balanced)."""
    base = (cap // n) & ~1
    sizes = [base] * n
    rem = cap - base * n
    i = 0
    while rem > 0:
        sizes[i % n] += 2
        rem -= 2
        i += 1
    out = []
    c0 = 0
    for cs in sizes:
        out.append((c0, cs))
        c0 += cs
    return out


def _build(cap):
    n_ch = -(-cap // 512)  # token chunks (<=512 fp32 psum free dim)
    chunks = _chunks(cap, n_ch)
    nch = len(chunks)
    # 2*nch PSUM banks are live in phase A (pg* + pu*); the chip has 8.
    assert nch <= 4, f"max expert load {cap} needs {nch} token chunks > 4"

    nc = bacc.Bacc(None, target_bir_lowering=False)
    x_d = nc.declare_dram_parameter("x", [P, D // P, cap], BF16, isOutput=False)
    gw_d = nc.declare_dram_parameter("gw", [P, F // P, D // P, P], BF16, isOutput=False)
    uw_d = nc.declare_dram_parameter("uw", [P, F // P, D // P, P], BF16, isOutput=False)
    dw_d = nc.declare_dram_parameter("dw", [P, F // P, D], BF16, isOutput=False)
    out_d = nc.declare_dram_parameter("out", [P, D // P, cap], BF16, isOutput=True)

    with tile.TileContext(nc) as tc:
        with (
            tc.tile_pool(name="deep", bufs=1) as deep,
            tc.tile_pool(name="wts", bufs=3) as wts,
            tc.tile_pool(name="stage", bufs=2) as stage,
            tc.tile_pool(name="ps", bufs=1, space="PSUM") as ps,
        ):
            wt_tiles = {}

            def load_ft(ft):
                gw_t = wts.tile([P, D // P, P], BF16, tag="gw", name="gw_t")
                nc.sync.dma_start(gw_t[:], gw_d[:, ft])
                uw_t = wts.tile([P, D // P, P], BF16, tag="uw", name="uw_t")
                nc.sync.dma_start(uw_t[:], uw_d[:, ft])
                wt_tiles[ft] = (gw_t, uw_t)

            # ft0 operands stream in PE consumption order (gate/up
            # interleaved per d-slice) as (gw, uw, x) 2-d-slice groups.
            gw0_t = wts.tile([P, D // P, P], BF16, tag="gw", name="gw0_t")
            uw0_t = wts.tile([P, D // P, P], BF16, tag="uw", name="uw0_t")
            x_t = deep.tile([P, D // P, cap], BF16, tag="x")
            # Parallel descriptor issue: weights on Sync, x on Scalar (the two
            # hardware-DGE queues) -- one queue alone takes ~0.6 us per
            # descriptor and would gate the ft0 ramp.
            for dt_ in range(0, D // P, 2):
                nc.sync.dma_start(gw0_t[:, ds(dt_, 2)], gw_d[:, 0, ds(dt_, 2)])
                nc.scalar.dma_start(x_t[:, ds(dt_, 2)], x_d[:, ds(dt_, 2)])
                nc.sync.dma_start(uw0_t[:, ds(dt_, 2)], uw_d[:, 0, ds(dt_, 2)])
            wt_tiles[0] = (gw0_t, uw0_t)
            load_ft(1)

            h_t = deep.tile([P, F // P, cap], BF16, tag="h")
            dw_t = deep.tile([P, F // P, D], BF16, tag="dw")

            # Phase A: h[fp, ft, c] = silu(g) * u; gate/up interleaved per dt
            for ft in range(F // P):
                if ft == 8:
                    for fo in range(0, F // P, 4):
                        nc.sync.dma_start(dw_t[:, ds(fo, 4)], dw_d[:, ds(fo, 4)])
                if ft + 1 < F // P and (ft + 1) not in wt_tiles:
                    load_ft(ft + 1)
                gw_t, uw_t = wt_tiles.pop(ft)
                pgs = [ps.tile([P, 512], F32, tag=f"pg{ci}", name=f"pg{ci}") for ci in range(nch)]
                pus = [ps.tile([P, 512], F32, tag=f"pu{ci}", name=f"pu{ci}") for ci in range(nch)]
                for dt_ in range(D // P):
                    for ci, (c0, cs) in enumerate(chunks):
                        nc.tensor.matmul(
                            pgs[ci][:, :cs], gw_t[:, dt_], x_t[:, dt_, ds(c0, cs)],
                            start=(dt_ == 0), stop=(dt_ == D // P - 1),
                        )
                    for ci, (c0, cs) in enumerate(chunks):
                        nc.tensor.matmul(
                            pus[ci][:, :cs], uw_t[:, dt_], x_t[:, dt_, ds(c0, cs)],
                            start=(dt_ == 0), stop=(dt_ == D // P - 1),
                        )
                sgs = []
                for ci, (c0, cs) in enumerate(chunks):
                    sg = stage.tile([P, 512], BF16, tag=f"sg{ci}", name=f"sg{ci}")
                    nc.scalar.activation(sg[:, :cs], pgs[ci][:, :cs],
                                         mybir.ActivationFunctionType.Silu)
                    sgs.append(sg)
                for ci, (c0, cs) in enumerate(chunks):
                    nc.vector.tensor_tensor(
                        h_t[:, ft, ds(c0, cs)], sgs[ci][:, :cs], pus[ci][:, :cs],
                        mybir.AluOpType.mult,
                    )

            # Phase B: outT[dp, dt, c] = sum_f dwT[f, d] * h[f, c].
            # Output banks alternate between the pg* and pu* tag sets so the
            # next d-tile's accumulation doesn't wait on this one's drain.
            for dt_ in range(D // P):
                grp = "pg" if dt_ % 2 == 0 else "pu"
                pos = [ps.tile([P, 512], F32, tag=f"{grp}{ci}", name=f"po{ci}") for ci in range(nch)]
                for fo in range(F // P):
                    for ci, (c0, cs) in enumerate(chunks):
                        nc.tensor.matmul(
                            pos[ci][:, :cs], dw_t[:, fo, ds(dt_ * P, P)],
                            h_t[:, fo, ds(c0, cs)],
                            start=(fo == 0), stop=(fo == F // P - 1),
                        )
                osb = stage.tile([P, cap], BF16, tag="osb", name="osb")
                for ci, (c0, cs) in enumerate(chunks):
                    if ci % 2 == 0:
                        nc.scalar.activation(osb[:, ds(c0, cs)], pos[ci][:, :cs],
                                             mybir.ActivationFunctionType.Copy)
                    else:
                        nc.vector.tensor_scalar_mul(osb[:, ds(c0, cs)], pos[ci][:, :cs], 1.0)
                nc.sync.dma_start(out_d[:, dt_], osb[:])
    nc.finalize()
    return nc


def _route(gating_output):
    """Numpy softmax + top-2 + renormalize; returns (ids [T,K], w [T,K])."""
    g = gating_output.astype(np.float32)
    m = g.max(axis=-1, keepdims=True)
    e = np.exp(g - m)
    probs = e / e.sum(axis=-1, keepdims=True)
    ids = np.argsort(-probs, axis=-1, kind="stable")[:, :TOPK]
    w = np.take_along_axis(probs, ids, axis=-1)
    w = w / w.sum(axis=-1, keepdims=True)
    return ids, w


def kernel(x, gating_output, gate_w, up_w, down_w):
    x = np.asarray(x, dtype=np.float32)
    gating_output = np.asarray(gating_output, dtype=np.float32)
    gate_w = np.asarray(gate_w, dtype=np.float32)
    up_w = np.asarray(up_w, dtype=np.float32)
    down_w = np.asarray(down_w, dtype=np.float32)

    ids, w = _route(gating_output)

    idx_e = []
    w_e = []
    for e in range(E):
        sel = np.nonzero((ids == e).any(axis=-1))[0]
        kpos = (ids[sel] == e).argmax(axis=-1)
        idx_e.append(sel)
        w_e.append(w[sel, kpos])

    cap = max(len(i) for i in idx_e)
    cap += cap & 1

    nc = _build(cap)

    in_maps = []
    for e in range(E):
        idx = idx_e[e]
        cnt = len(idx)
        x_pad = np.zeros((cap, D), dtype=np.float32)
        x_pad[:cnt] = x[idx]

        # x: [cap, D] -> [128(dp), D/128(do), cap]
        x_dev = np.ascontiguousarray(
            x_pad.T.reshape(D // P, P, cap).transpose(1, 0, 2)).astype(BF)
        # gate/up: [F, D] -> T -> [D, F] -> [128(dp), 16(ft), 8(do), 128(fi)]
        gwT = gate_w[e].T  # [D, F]
        gw_dev = np.ascontiguousarray(
            gwT.reshape(D // P, P, F // P, P).transpose(1, 2, 0, 3)).astype(BF)
        uwT = up_w[e].T
        uw_dev = np.ascontiguousarray(
            uwT.reshape(D // P, P, F // P, P).transpose(1, 2, 0, 3)).astype(BF)
        # down: [D, F] -> T -> [F, D] -> [128(fp), 16(fo), D]
        dwT = down_w[e].T  # [F, D]
        dw_dev = np.ascontiguousarray(
            dwT.reshape(F // P, P, D).transpose(1, 0, 2)).astype(BF)

        in_maps.append({"x": x_dev, "gw": gw_dev, "uw": uw_dev, "dw": dw_dev})

    def _run():
        try:
            return run_bass_kernel_spmd(nc, in_maps, core_ids=list(range(N_CORES)))
        except Exception:
            # First execution of a fresh NEFF occasionally dies with
            # NRT_EXEC_UNIT_UNRECOVERABLE on this setup; the retry reuses
            # the cached executable and goes through.
            import time as _time

            _time.sleep(5)
            return run_bass_kernel_spmd(nc, in_maps, core_ids=list(range(N_CORES)))

    def _assemble(res):
        out = np.zeros((T, D), dtype=np.float32)
        for e in range(E):
            cnt = len(idx_e[e])
            # device out: [dp, do, c] -> [c, do*128+dp]
            o = res.results[e]["out"].astype(np.float32).transpose(2, 1, 0).reshape(cap, D)
            out[idx_e[e]] += o[:cnt] * w_e[e][:, None]
        return out

    def _spot_check(out):
        # One token per (non-empty) expert, host-computed in f32. Catches the
        # rare corrupted execution (seen once: silently wrong rows on a fresh
        # NEFF) -- bf16 rounding keeps honest rows well under the threshold.
        worst = 0.0
        for e in range(E):
            if len(idx_e[e]) == 0:
                continue
            t = int(idx_e[e][0])
            acc = np.zeros(D, dtype=np.float32)
            for k in range(TOPK):
                ek = int(ids[t, k])
                g = gate_w[ek] @ x[t]
                u = up_w[ek] @ x[t]
                h = (g / (1.0 + np.exp(-g))) * u
                acc += w[t, k] * (down_w[ek] @ h)
            scale = np.abs(acc).max() + 1e-6
            worst = max(worst, np.abs(out[t] - acc).max() / scale)
        return worst

    res = _run()
    out = _assemble(res)
    if _spot_check(out) > 0.1:
        res = _run()
        out = _assemble(res)
    return out


# revision 3
# speedup vs baseline: 1.0025x; 1.0025x over previous
"""Dense-MoE (top-2 of 8 experts) TRN2 kernel v4: expert-parallel, bf16 matmuls.

Host side: softmax + top-2 routing, per-expert token gather (padded to the max
expert load), weight re-layout + bf16 conversion. Device side (per core = one
expert), all matmul operands bf16, PSUM accumulation f32:
    phase A:  h[f, c] = silu(gw @ x) * (uw @ x)      [f-major, bf16 in SBUF]
    phase B:  outT[d, c] = sum_f dwT[f, d] * h[f, c]  [tokens on the free dim]
The routing weight and the scatter-add back to [T, D] happen on the host
(out[t] += w_t * outT[:, c].T), so the device kernel needs no tw input.

vs v2: gate/up matmuls interleave per d-slice so the first f-tile's x
streaming window doubles (the gate-only pass needs 2.9 MB in 4.3 us, over
HBM rate; interleaved it is 3.1 MB in 8.6 us); the startup DMAs issue in PE
consumption order as (gw 2-slice, uw 2-slice, x 2-slice) groups, all on the
Sync queue (GpSimd-issued DMAs land on a software-dynamic DGE queue with
~1.6 us drains -- tried in v4, slower); phase B PSUM banks alternate tag
sets so consecutive d-tiles don't wait on the drain, and the drain copies
alternate scalar/vector so the final d-tile empties ~2 us sooner. fp16 was
tried and is ~25% slower than bf16 on the PE despite the cost model
claiming parity.
"""
import sys

sys.path.insert(0, "/opt/trn_rl_repo")

import ml_dtypes
import numpy as np

import concourse.bass as bass
from concourse import bacc
import concourse.mybir as mybir
import concourse.tile as tile
from concourse.bass_utils import run_bass_kernel_spmd
from concourse.bass import ds

T, D, F, E, TOPK = 4096, 1024, 2048, 8, 2
P = 128
N_CORES = 8

F32 = mybir.dt.float32
BF16 = mybir.dt.bfloat16
BF = ml_dtypes.bfloat16


def _chunks(cap, n):
    """Split [0, cap) into n even-sized chunks (sizes even, ~balanced)."""
    base = (cap // n) & ~1
    sizes = [base] * n
    rem = cap - base * n
    i = 0
    while rem > 0:
        sizes[i % n] += 2
        rem -= 2
        i += 1
    out = []
    c0 = 0
    for cs in sizes:
        out.append((c0, cs))
        c0 += cs
    return out


def _build(cap):
    n_ch = -(-cap // 512)  # token chunks (<=512 fp32 psum free dim)
    chunks = _chunks(cap, n_ch)
    nch = len(chunks)
    # 2*nch PSUM banks are live in phase A (pg* + pu*); the chip has 8.
    assert nch <= 4, f"max expert load {cap} needs {nch} token chunks > 4"

    nc = bacc.Bacc(None, target_bir_lowering=False)
    x_d = nc.declare_dram_parameter("x", [P, D // P, cap], BF16, isOutput=False)
    gw_d = nc.declare_dram_parameter("gw", [P, F // P, D // P, P], BF16, isOutput=False)
    uw_d = nc.declare_dram_parameter("uw", [P, F // P, D // P, P], BF16, isOutput=False)
    dw_d = nc.declare_dram_parameter("dw", [P, F // P, D], BF16, isOutput=False)
    out_d = nc.declare_dram_parameter("out", [P, D // P, cap], BF16, isOutput=True)

    with tile.TileContext(nc) as tc:
        with (
            tc.tile_pool(name="deep", bufs=1) as deep,
            tc.tile_pool(name="wts", bufs=3) as wts,
            tc.tile_pool(name="stage", bufs=2) as stage,
            tc.tile_pool(name="ps", bufs=1, space="PSUM") as ps,
        ):
            wt_tiles = {}

            def load_ft(ft):
                gw_t = wts.tile([P, D // P, P], BF16, tag="gw", name="gw_t")
                nc.sync.dma_start(gw_t[:], gw_d[:, ft])
                uw_t = wts.tile([P, D // P, P], BF16, tag="uw", name="uw_t")
                nc.sync.dma_start(uw_t[:], uw_d[:, ft])
                wt_tiles[ft] = (gw_t, uw_t)

            # ft0 operands stream in PE consumption order (gate/up
            # interleaved per d-slice) as (gw, uw, x) 2-d-slice groups.
            gw0_t = wts.tile([P, D // P, P], BF16, tag="gw", name="gw0_t")
            uw0_t = wts.tile([P, D // P, P], BF16, tag="uw", name="uw0_t")
            x_t = deep.tile([P, D // P, cap], BF16, tag="x")
            # Parallel descriptor issue: weights on Sync, x on Scalar (the two
            # hardware-DGE queues) -- one queue alone takes ~0.6 us per
            # descriptor and would gate the ft0 ramp.
            for dt_ in range(0, D // P, 2):
                nc.sync.dma_start(gw0_t[:, ds(dt_, 2)], gw_d[:, 0, ds(dt_, 2)])
                nc.scalar.dma_start(x_t[:, ds(dt_, 2)], x_d[:, ds(dt_, 2)])
                nc.sync.dma_start(uw0_t[:, ds(dt_, 2)], uw_d[:, 0, ds(dt_, 2)])
            wt_tiles[0] = (gw0_t, uw0_t)
            load_ft(1)

            h_t = deep.tile([P, F // P, cap], BF16, tag="h")
            dw_t = deep.tile([P, F // P, D], BF16, tag="dw")

            # Phase A: h[fp, ft, c] = silu(g) * u; gate/up interleaved per dt
            for ft in range(F // P):
                if ft == 8:
                    for fo in range(0, F // P, 4):
                        nc.sync.dma_start(dw_t[:, ds(fo, 4)], dw_d[:, ds(fo, 4)])
                if ft + 1 < F // P and (ft + 1) not in wt_tiles:
                    load_ft(ft + 1)
                gw_t, uw_t = wt_tiles.pop(ft)
                pgs = [ps.tile([P, 512], F32, tag=f"pg{ci}", name=f"pg{ci}") for ci in range(nch)]
                pus = [ps.tile([P, 512], F32, tag=f"pu{ci}", name=f"pu{ci}") for ci in range(nch)]
                for dt_ in range(D // P):
                    for ci, (c0, cs) in enumerate(chunks):
                        nc.tensor.matmul(
                            pgs[ci][:, :cs], gw_t[:, dt_], x_t[:, dt_, ds(c0, cs)],
                            start=(dt_ == 0), stop=(dt_ == D // P - 1),
                        )
                    for ci, (c0, cs) in enumerate(chunks):
                        nc.tensor.matmul(
                            pus[ci][:, :cs], uw_t[:, dt_], x_t[:, dt_, ds(c0, cs)],
                            start=(dt_ == 0), stop=(dt_ == D // P - 1),
                        )
                sgs = []
                for ci, (c0, cs) in enumerate(chunks):
                    sg = stage.tile([P, 512], BF16, tag=f"sg{ci}", name=f"sg{ci}")
                    nc.scalar.activation(sg[:, :cs], pgs[ci][:, :cs],
                                         mybir.ActivationFunctionType.Silu)
                    sgs.append(sg)
                for ci, (c0, cs) in enumerate(chunks):
                    nc.vector.tensor_tensor(
                        h_t[:, ft, ds(c0, cs)], sgs[ci][:, :cs], pus[ci][:, :cs],
                        mybir.AluOpType.mult,
                    )

            # Phase B: outT[dp, dt, c] = sum_f dwT[f, d] * h[f, c].
            # Output banks alternate between the pg* and pu* tag sets so the
            # next d-tile's accumulation doesn't wait on this one's drain.
            for dt_ in range(D // P):
                grp = "pg" if dt_ % 2 == 0 else "pu"
                pos = [ps.tile([P, 512], F32, tag=f"{grp}{ci}", name=f"po{ci}") for ci in range(nch)]
                last = dt_ == D // P - 1
                if last:
                    # Sequential chunks: each chunk's accumulation group
                    # closes at 1/3 intervals, so its drain copy + DMA
                    # overlap the remaining chunks' matmuls instead of all
                    # stacking after the final matmul of the kernel.
                    for ci, (c0, cs) in enumerate(chunks):
                        for fo in range(F // P):
                            nc.tensor.matmul(
                                pos[ci][:, :cs], dw_t[:, fo, ds(dt_ * P, P)],
                                h_t[:, fo, ds(c0, cs)],
                                start=(fo == 0), stop=(fo == F // P - 1),
                            )
                else:
                    for fo in range(F // P):
                        for ci, (c0, cs) in enumerate(chunks):
                            nc.tensor.matmul(
                                pos[ci][:, :cs], dw_t[:, fo, ds(dt_ * P, P)],
                                h_t[:, fo, ds(c0, cs)],
                                start=(fo == 0), stop=(fo == F // P - 1),
                            )
                osb = stage.tile([P, cap], BF16, tag="osb", name="osb")
                for ci, (c0, cs) in enumerate(chunks):
                    if ci % 2 == 0:
                        nc.scalar.activation(osb[:, ds(c0, cs)], pos[ci][:, :cs],
                                             mybir.ActivationFunctionType.Copy)
                    else:
                        nc.vector.tensor_scalar_mul(osb[:, ds(c0, cs)], pos[ci][:, :cs], 1.0)
                if last:
                    # Per-chunk DMA: subtile deps release each slice as soon
                    # as its copy lands.
                    for ci, (c0, cs) in enumerate(chunks):
                        nc.sync.dma_start(out_d[:, dt_, ds(c0, cs)], osb[:, ds(c0, cs)])
                else:
                    nc.sync.dma_start(out_d[:, dt_], osb[:])
    nc.finalize()
    return nc


def _route(gating_output):
    """Numpy softmax + top-2 + renormalize; returns (ids [T,K], w [T,K])."""
    g = gating_output.astype(np.float32)
    m = g.max(axis=-1, keepdims=True)
    e = np.exp(g - m)
    probs = e / e.sum(axis=-1, keepdims=True)
    ids = np.argsort(-probs, axis=-1, kind="stable")[:, :TOPK]
    w = np.take_along_axis(probs, ids, axis=-1)
    w = w / w.sum(axis=-1, keepdims=True)
    return ids, w


def kernel(x, gating_output, gate_w, up_w, down_w):
    x = np.asarray(x, dtype=np.float32)
    gating_output = np.asarray(gating_output, dtype=np.float32)
    gate_w = np.asarray(gate_w, dtype=np.float32)
    up_w = np.asarray(up_w, dtype=np.float32)
    down_w = np.asarray(down_w, dtype=np.float32)

    ids, w = _route(gating_output)

    idx_e = []
    w_e = []
    for e in range(E):
        sel = np.nonzero((ids == e).any(axis=-1))[0]
        kpos = (ids[sel] == e).argmax(axis=-1)
        idx_e.append(sel)
        w_e.append(w[sel, kpos])

    cap = max(len(i) for i in idx_e)
    cap += cap & 1

    nc = _build(cap)

    in_maps = []
    for e in range(E):
        idx = idx_e[e]
        cnt = len(idx)
        x_pad = np.zeros((cap, D), dtype=np.float32)
        x_pad[:cnt] = x[idx]

        # x: [cap, D] -> [128(dp), D/128(do), cap]
        x_dev = np.ascontiguousarray(
            x_pad.T.reshape(D // P, P, cap).transpose(1, 0, 2)).astype(BF)
        # gate/up: [F, D] -> T -> [D, F] -> [128(dp), 16(ft), 8(do), 128(fi)]
        gwT = gate_w[e].T  # [D, F]
        gw_dev = np.ascontiguousarray(
            gwT.reshape(D // P, P, F // P, P).transpose(1, 2, 0, 3)).astype(BF)
        uwT = up_w[e].T
        uw_dev = np.ascontiguousarray(
            uwT.reshape(D // P, P, F // P, P).transpose(1, 2, 0, 3)).astype(BF)
        # down: [D, F] -> T -> [F, D] -> [128(fp), 16(fo), D]
        dwT = down_w[e].T  # [F, D]
        dw_dev = np.ascontiguousarray(
            dwT.reshape(F // P, P, D).transpose(1, 0, 2)).astype(BF)

        in_maps.append({"x": x_dev, "gw": gw_dev, "uw": uw_dev, "dw": dw_dev})

    def _run():
        try:
            return run_bass_kernel_spmd(nc, in_maps, core_ids=list(range(N_CORES)))
        except Exception:
            # First execution of a fresh NEFF occasionally dies with
            # NRT_EXEC_UNIT_UNRECOVERABLE on this setup; the retry reuses
            # the cached executable and goes through.
            import time as _time

            _time.sleep(5)
            return run_bass_kernel_spmd(nc, in_maps, core_ids=list(range(N_CORES)))

    def _assemble(res):
        out = np.zeros((T, D), dtype=np.float32)
        for e in range(E):
            cnt = len(idx_e[e])
            # device out: [dp, do, c] -> [c, do*128+dp]
            o = res.results[e]["out"].astype(np.float32).transpose(2, 1, 0).reshape(cap, D)
            out[idx_e[e]] += o[:cnt] * w_e[e][:, None]
        return out

    def _spot_check(out):
        # One token per (non-empty) expert, host-computed in f32. Catches the
        # rare corrupted execution (seen once: silently wrong rows on a fresh
        # NEFF) -- bf16 rounding keeps honest rows well under the threshold.
        worst = 0.0
        for e in range(E):
            if len(idx_e[e]) == 0:
                continue
            t = int(idx_e[e][0])
            acc = np.zeros(D, dtype=np.float32)
            for k in range(TOPK):
                ek = int(ids[t, k])
                g = gate_w[ek] @ x[t]
                u = up_w[ek] @ x[t]
                h = (g / (1.0 + np.exp(-g))) * u
                acc += w[t, k] * (down_w[ek] @ h)
            scale = np.abs(acc).max() + 1e-6
            worst = max(worst, np.abs(out[t] - acc).max() / scale)
        return worst

    res = _run()
    out = _assemble(res)
    if _spot_check(out) > 0.1:
        res = _run()
        out = _assemble(res)
    return out
